# revision 53
# baseline (speedup 1.0000x reference)
"""Trainium2 Bass kernel for the EnsembleFeatureLoss OT problem.

Math (per ensemble member e of E=4):
  s = l2norm_rows(gts[e]); t = l2norm_rows(feats[e])      # [4096, 1024]
  sim = s @ t.T                                            # [4096, 4096]
  K = exp(10*sim - 10)
  Sinkhorn converges in exactly 2 iterations for this regime (verified
  against the reference; re-verified on the host from kernel outputs,
  with a full numpy fallback if that check ever fails):
    r1 = u / rowsum(K);  c1 = v / (K.T @ r1)
    r2 = u / (K @ c1);   c2 = v / (K.T @ r2)
  loss_e = sum(outer(r2, c2) * K * sim) = sum_n c2[n] * Z[n]

Distribution: 8 cores = 4 members x 2 row-halves (2048 rows each).

Single-pass structure (v3): one fused pass over the [2048, 4096] block
computes sim' chunks on the PE (fp8 DoubleRow), exponentiates ONCE on
the scalar engine writing K8 = exp(10*sim) to SBUF as fp8 (rowsum rides
the ACT accumulator), and the Z-ingredient G = 0.25*sim'*K8 is formed
from PSUM by DVE/GpSimd scalar_tensor_tensor, also fp8 in SBUF. Nothing
spills to DRAM. All column reductions (Y1 = K8^T r1, Y2 = K8^T r2,
ZG = G^T r2) are DoubleRow fp8 matmuls with per-row-tile weights into
single PSUM banks. Sinkhorn's pair exchange is one 16KB AllReduce of
Y1. Pass B computes rowdot = K8 @ c1 via DVE scalar_tensor_tensor
with accumulator, then r2, then the Y2/ZG colsum matmuls (Y2 in psum
row 0, ZG in row 1 of the same banks via two weight variants).

Scale bookkeeping: operands are prescaled by SIGMA=16 (sim' = 256*sim);
K8 = exp(10*sim) = K*e^10 keeps fp8 range happy; r1/r2 are computed
with u*e^10 so they equal the TRUE Sinkhorn scalings; c1 uses v*e^10 so
kc = K8*c1 is O(1). The e^10 factors cancel in the host combine:
  loss_e = sum_n v * ZG[n] / (64 * Y2[n]).
"""

import numpy as np
import ml_dtypes

BF16 = ml_dtypes.bfloat16
F8 = ml_dtypes.float8_e4m3
SIGMA = 16.0                # fp8 operand pre-scale; sim' = SIGMA^2 * sim
E10 = float(np.float32(np.exp(10.0)))

E = 4
M = 4096
N = 4096
D = 1024
P = 128
NCORES = 8
MHALF = M // 2              # rows per core
CH = 512                    # one fp32 psum bank
UW = 1024                   # pass-A psum rotation unit width (2 banks)

_CACHE = {}


def build_bass(mhalf=MHALF, n=N, d=D, ncores=NCORES, m_total=None):
    import concourse.bass as bass
    import concourse.mybir as mybir
    import concourse.tile as tile
    from concourse import bacc
    from concourse.bass import ts

    dt = mybir.dt
    f32, bf16, f8 = dt.float32, dt.bfloat16, dt.float8e4
    Alu = mybir.AluOpType
    Act = mybir.ActivationFunctionType

    if m_total is None:
        m_total = 2 * mhalf
    nt_m = mhalf // P           # 16 row tiles
    npair = nt_m // 2           # 8 tile pairs
    nd2 = d // 256              # 4 DoubleRow contraction blocks
    nch = n // CH               # 8 column chunks of 512
    nun = n // UW               # 4 rotation units per tile
    W_SC = 512.0                # fp8 weight prescale (cancels on host)
    u_sc = float(np.float32(E10 / m_total))   # u * e^10
    uw_sc = float(np.float32(E10 / m_total * W_SC))
    v_sc = float(np.float32(E10 / n * W_SC))  # v * e^10 * W_SC
    rg = [[i, i + 1] for i in range(0, ncores, 2)]

    # pass-B engine assignment: which tiles' rowdot runs on gpsimd
    GP_B = {1, 3, 5, 7, 11}           # 5 tiles on gpsimd, 11 on DVE
    # pass-A G runs on DVE only (GPSIMD cannot read PSUM)
    GP_A = set()

    nc = bacc.Bacc("TRN2", target_bir_lowering=False, debug=False,
                   num_devices=ncores)
    sT = nc.declare_dram_parameter("sT", [nd2, P, 2, mhalf], f8, isOutput=False)
    tT = nc.declare_dram_parameter("tT", [nd2, P, 2, n], f8, isOutput=False)
    vecs = nc.declare_dram_parameter("vecs", [2, n], f32, isOutput=True)
    r1o = nc.declare_dram_parameter("r1o", [P, nt_m], f32, isOutput=True)
    r2o = nc.declare_dram_parameter("r2o", [P, nt_m], f32, isOutput=True)

    with tile.TileContext(nc) as tc:
        with (
            tc.tile_pool(name="persist", bufs=1) as pp,
            tc.tile_pool(name="k8", bufs=npair) as k8p,      # K8 pair tiles
            tc.tile_pool(name="gg", bufs=npair) as ggp,      # G pair tiles
            tc.tile_pool(name="tin", bufs=nd2) as tinp,      # tT blocks
            tc.tile_pool(name="sin", bufs=nd2) as sinp,      # sT blocks
            tc.tile_pool(name="dump", bufs=1) as dumpp,
            tc.tile_pool(name="dumpg", bufs=2) as dumpgp,      # rowdot dumps
            tc.tile_pool(name="vec", bufs=2) as vecp,        # [8,512]-ish f32
            tc.tile_pool(name="sm", bufs=12) as smp,
            tc.tile_pool(name="smg", bufs=3) as smgp,         # tiny stats
            tc.tile_pool(name="ps", bufs=3, space="PSUM") as psp,   # 6 banks
            tc.tile_pool(name="py", bufs=1, space="PSUM") as pyp,   # 2 banks
            tc.tile_pool(name="dram", bufs=1, space="DRAM") as dp,
        ):
            # ---- dram scratch ----
            y1_in = dp.tile([1, n], f32, name="y1_in", tag="y1_in")
            y1_out = dp.tile([1, n], f32, name="y1_out", tag="y1_out")
            c1_d = dp.tile([1, n], f8, name="c1_d", tag="c1_d")

            # ---- persistent sbuf ----
            tTb = [tinp.tile([P, 2, n], f8, name=f"tTb{b}", tag="tin")
                   for b in range(nd2)]
            sTb = [sinp.tile([P, 2, mhalf], f8, name=f"sTb{b}", tag="sin")
                   for b in range(nd2)]
            K8 = [k8p.tile([P, 2, n], f8, name=f"K8_{p}", tag="k8")
                  for p in range(npair)]
            GG = [ggp.tile([P, 2, n], f8, name=f"GG_{p}", tag="gg")
                  for p in range(npair)]
            c1_bc = pp.tile([P, n], f8, name="c1_bc", tag="c1_bc")
            # colsum weights: [part, dr-pair, pair, chunk, col8]; variant c
            # holds the r-pair in col c and zeros elsewhere, so each chunk
            # matmul writes [8, 512] of ONE psum bank at base partition 0:
            # row c accumulates the real chunk-c colsum, other rows += 0.
            w1 = pp.tile([P, 2, npair, nch, nch], f8, name="w1", tag="w1")
            # w2 variant v holds r2 in col v: variant 0 drives Y2 into psum
            # row 0, variant 1 drives ZG into row 1 of the same banks.
            w2 = pp.tile([P, 2, npair, 2, 2], f8, name="w2", tag="w2")
            r1buf = pp.tile([P, nt_m], f32, name="r1buf", tag="r1buf")
            r2buf = pp.tile([P, nt_m], f32, name="r2buf", tag="r2buf")
            scale10 = pp.tile([P, 1], f32, name="scale10", tag="scale10")
            bias0 = pp.tile([P, 1], f32, name="bias0", tag="bias0")

            nc.vector.memset(scale10[:], 10.0 / (SIGMA * SIGMA))
            nc.vector.memset(bias0[:], 0.0)
            nc.vector.memset(w1[:], 0.0)
            nc.vector.memset(w2[:], 0.0)

            # ---- input loads: all on the sync queue (the gpsimd queue
            # must stay empty so the collective barrier isn't delayed),
            # column-chunked so the first tiles' operands land first ----
            def _load_t(cg):
                for b in range(nd2):
                    nc.sync.dma_start(
                        tTb[b][:, :, ts(cg, n // 4)],
                        tT[b][:, :, ts(cg, n // 4)])
            def _load_s(cg):
                for b in range(nd2):
                    nc.sync.dma_start(
                        sTb[b][:, :, ts(cg, mhalf // 4)],
                        sT[b][:, :, ts(cg, mhalf // 4)])
            _load_t(0); _load_s(0); _load_t(1); _load_t(2); _load_t(3)
            _load_s(1); _load_s(2); _load_s(3)

            # ---- Y1 colsum accumulator: one bank, row c = chunk c ----
            y1b = pyp.tile([P, CH], f32, name="y1b", tag="py")

            # ---- pass A ----
            for mi in range(nt_m):
                pr, half = divmod(mi, 2)
                rs = smp.tile([P, nun], f32, name="rs", tag="sm")
                for u in range(nun):
                    pm = psp.tile([P, UW], f32, name="pm", tag="ps")
                    for dd in range(nd2):
                        for q in range(UW // CH):
                            nc.tensor.matmul(
                                pm[:, ts(q, CH)],
                                sTb[dd][:, :, ts(mi, P)],
                                tTb[dd][:, :, ts(u * 2 + q, CH)],
                                start=(dd == 0), stop=(dd == nd2 - 1),
                                perf_mode=mybir.MatmulPerfMode.DoubleRow)
                    nc.scalar.activation(
                        K8[pr][:, half, ts(u, UW)], pm[:], Act.Exp,
                        bias=bias0[:], scale=scale10[:],
                        accum_out=rs[:, u:u + 1])
                    if mi in GP_A:
                        nc.gpsimd.scalar_tensor_tensor(
                            out=GG[pr][:, half, ts(u, UW)], in0=pm[:],
                            scalar=0.25, in1=K8[pr][:, half, ts(u, UW)],
                            op0=Alu.mult, op1=Alu.mult)
                    else:
                        nc.vector.scalar_tensor_tensor(
                            out=GG[pr][:, half, ts(u, UW)], in0=pm[:],
                            scalar=0.25, in1=K8[pr][:, half, ts(u, UW)],
                            op0=Alu.mult, op1=Alu.mult)
                rowsum = smp.tile([P, 1], f32, name="rowsum", tag="sm")
                nc.vector.tensor_reduce(rowsum[:], rs[:],
                                        mybir.AxisListType.X, Alu.add)
                rinv = smp.tile([P, 1], f32, name="rinv", tag="sm")
                nc.vector.reciprocal(rinv[:], rowsum[:])
                nc.vector.tensor_scalar_mul(r1buf[:, mi:mi + 1], rinv[:], u_sc)
                for c in range(nch):
                    nc.vector.tensor_scalar_mul(
                        w1[:, half, pr, c, c:c + 1], rinv[:], uw_sc)
                if half == 1:
                    # Y1 += r1-pair^T . K8-pair per column chunk
                    for c in range(nch):
                        nc.tensor.matmul(
                            y1b[0:nch, :],
                            w1[:, :, pr, c, :],
                            K8[pr][:, :, ts(c, CH)],
                            start=(pr == 0 and c == 0),
                            stop=(pr == npair - 1 and c == nch - 1),
                            perf_mode=mybir.MatmulPerfMode.DoubleRow,
                            skip_group_check=True)

            nc.sync.dma_start(r1o[:, :], r1buf[:])

            # ---- Y1 -> AllReduce -> c1 (scaled by v*e^10*W_SC) ----
            y1sb = vecp.tile([nch, CH], f32, name="y1sb", tag="vec")
            nc.vector.tensor_copy(y1sb[:], y1b[0:nch, :])
            nc.gpsimd.dma_start(
                y1_in[0:1, :].rearrange("a (c f) -> (a c) f", c=nch),
                y1sb[:])
            nc.gpsimd.collective_compute(
                "AllReduce", Alu.add, replica_groups=rg,
                ins=[y1_in.opt()], outs=[y1_out.opt()])
            y1l = vecp.tile([64, 64], f32, name="y1l", tag="vec")
            nc.gpsimd.dma_start(
                y1l[:], y1_out[0:1, :].rearrange("a (c f) -> (a c) f", c=64))
            nc.vector.reciprocal(y1l[:], y1l[:])
            c1s = vecp.tile([64, 64], f8, name="c1s", tag="vec")
            nc.vector.tensor_scalar_mul(c1s[:], y1l[:], v_sc)
            nc.gpsimd.dma_start(
                c1_d[0:1, :].rearrange("a (c f) -> (a c) f", c=64), c1s[:])
            nc.scalar.dma_start(c1_bc[:], c1_d[0:1, :].to_broadcast((P, n)))

            # ---- pass B ----
            # Y2 (psum row 0) and ZG (psum row 32) accumulate per column
            # chunk in per-chunk banks reusing the freed pass-A rotation
            # buffers: chunk c -> bank-tile T[c//2], column half c%2.
            yz = [psp.tile([P, UW], f32, name=f"yz{t}", tag="ps")
                  for t in range(3)]
            yz.append(pyp.tile([P, UW], f32, name="yz3", tag="py"))
            # rowdot engine split: these tiles' kc runs as V TT + scalar
            # accum-copy; the rest use a single V STT with accumulator.
            # hybrid rowdots: odd processing positions split columns
            # V(0:2048) / GP+S(2048:4096); the combining add is deferred
            # one tile so the in-order V queue never waits on GP/S.
            order_b = [0, 1, 14, 15, 2, 3, 4, 5, 6, 7, 8, 9, 10, 11,
                       12, 13]
            nh = n // 2
            done_pairs = 0
            backed = set()

            def _back(mi, pr, half, rowdot):
                nonlocal done_pairs
                rdinv = smp.tile([P, 1], f32, name="rdinv", tag="sm")
                nc.vector.reciprocal(rdinv[:], rowdot[:])
                nc.scalar.mul(r2buf[:, mi:mi + 1], rdinv[:], u_sc)
                for v in range(2):
                    nc.scalar.mul(w2[:, half, pr, v, v:v + 1], rdinv[:],
                                  uw_sc)
                backed.add(mi)
                if (pr * 2) in backed and (pr * 2 + 1) in backed:
                    done_pairs += 1
                    for c in range(nch):
                        nc.tensor.matmul(
                            yz[c // 2][0:2, ts(c % 2, CH)],
                            w2[:, :, pr, 0, :],
                            K8[pr][:, :, ts(c, CH)],
                            start=(done_pairs == 1), stop=False,
                            perf_mode=mybir.MatmulPerfMode.DoubleRow,
                            skip_group_check=True)
                        nc.tensor.matmul(
                            yz[c // 2][0:2, ts(c % 2, CH)],
                            w2[:, :, pr, 1, :],
                            GG[pr][:, :, ts(c, CH)],
                            start=False, stop=(done_pairs == npair),
                            perf_mode=mybir.MatmulPerfMode.DoubleRow,
                            skip_group_check=True)

            pend = None          # deferred GP-split tile
            for idx, mi in enumerate(order_b):
                pr, half = divmod(mi, 2)
                if idx % 2 == 1:
                    # split tile: V half + GP/S half, combine deferred
                    rdA = smp.tile([P, 1], f32, name="rdA", tag="sm")
                    rdB = smgp.tile([P, 1], f32, name="rdB", tag="smg")
                    kcd = tinp.tile([P, nh], f8, name="kcd", tag="tin")
                    nc.vector.scalar_tensor_tensor(
                        out=kcd[:], in0=K8[pr][:, half, 0:nh], scalar=1.0,
                        in1=c1_bc[:, 0:nh], op0=Alu.mult, op1=Alu.mult,
                        accum_out=rdA[:])
                    kcg = dumpgp.tile([P, nh], bf16, name="kcg", tag="dumpg")
                    nc.gpsimd.tensor_tensor(kcg[:], K8[pr][:, half, nh:n],
                                            c1_bc[:, nh:n], Alu.mult)
                    nc.scalar.activation(kcg[:], kcg[:], Act.Copy,
                                         accum_out=rdB[:])
                    pend = (mi, pr, half, rdA, rdB)
                    continue
                rowdot = smp.tile([P, 1], f32, name="rowdot", tag="sm")
                kcd = tinp.tile([P, n], f8, name="kcd", tag="tin")
                nc.vector.scalar_tensor_tensor(
                    out=kcd[:], in0=K8[pr][:, half, :], scalar=1.0,
                    in1=c1_bc[:], op0=Alu.mult, op1=Alu.mult,
                    accum_out=rowdot[:])
                _back(mi, pr, half, rowdot)
                if pend is not None:
                    gmi, gpr, ghalf, rdA, rdB = pend
                    pend = None
                    grd = smp.tile([P, 1], f32, name="grd", tag="sm")
                    nc.vector.tensor_add(grd[:], rdA[:], rdB[:])
                    _back(gmi, gpr, ghalf, grd)
            if pend is not None:
                gmi, gpr, ghalf, rdA, rdB = pend
                grd = smp.tile([P, 1], f32, name="grd", tag="sm")
                nc.vector.tensor_add(grd[:], rdA[:], rdB[:])
                _back(gmi, gpr, ghalf, grd)

            nc.sync.dma_start(r2o[:, :], r2buf[:])

            # ---- outputs ----
            yzsb = vecp.tile([P, UW], f32, name="yzsb", tag="vec")
            for t in range(4):
                if t % 2 == 0:
                    nc.vector.tensor_copy(yzsb[32 * t:32 * t + 2, :],
                                          yz[t][0:2, :])
                else:
                    nc.scalar.copy(yzsb[32 * t:32 * t + 2, :],
                                   yz[t][0:2, :])
            for t in range(4):
                nc.sync.dma_start(vecs[0:1, ts(t, UW)],
                                  yzsb[32 * t:32 * t + 1, :])
                nc.sync.dma_start(vecs[1:2, ts(t, UW)],
                                  yzsb[32 * t + 1:32 * t + 2, :])

    return nc


def _normalize_rows(x):
    x = np.asarray(x, np.float32)
    nrm = np.sqrt((x * x).sum(axis=1, keepdims=True))
    return x / np.maximum(nrm, 1e-12)


def _pair_pack(xT):
    """[D, C] -> [D//256, 128, 2, C] DoubleRow operand layout (fp8)."""
    Dd, C = xT.shape
    return np.ascontiguousarray(
        xT.reshape(Dd // 256, 2, P, C).transpose(0, 2, 1, 3))


def _make_in_maps(gts, feats):
    in_maps = []
    sn = [_normalize_rows(gts[e]) for e in range(E)]
    tn8 = [_pair_pack((SIGMA * _normalize_rows(feats[e]).T).astype(F8))
           for e in range(E)]
    for core in range(NCORES):
        e, h = divmod(core, 2)
        s_half = sn[e][h * MHALF:(h + 1) * MHALF]          # [2048, 1024]
        in_maps.append({
            "sT": _pair_pack((SIGMA * s_half.T).astype(F8)),
            "tT": tn8[e],
        })
    return in_maps


def _ensemble(losses, prev_losses):
    l = np.asarray(losses, np.float64)
    ratio = l / (np.asarray(prev_losses, np.float64) + 1e-8)
    w = np.exp(ratio / 1.0)
    w = w / np.sum(w) * l.shape[0]
    return np.float32(np.sum(w * l))


def _numpy_reference(gts, feats, prev_losses):
    """Faithful float32 fallback, used only if the on-device convergence
    check is violated (never observed for this problem's regime)."""
    losses = []
    for e in range(gts.shape[0]):
        s = gts[e] / np.maximum(
            np.linalg.norm(gts[e], axis=1, keepdims=True), 1e-12)
        t = feats[e] / np.maximum(
            np.linalg.norm(feats[e], axis=1, keepdims=True), 1e-12)
        sim = (s @ t.T).astype(np.float32)
        K = np.exp(-(1.0 - sim) / 0.1)
        m, n = sim.shape
        u = np.full(m, 1.0 / m, np.float32)
        v = np.full(n, 1.0 / n, np.float32)
        r = np.ones(m, np.float32)
        c = np.ones(n, np.float32)
        err = np.inf
        for _ in range(100):
            if err < 0.01:
                break
            r_new = u / (K @ c)
            c = v / (K.T @ r_new)
            err = float(np.mean(np.abs(r_new - r)))
            r = r_new
        losses.append(np.sum(np.outer(r, c) * K * sim))
    return _ensemble(losses, prev_losses)


def _run(gts, feats, trace=False):
    from concourse.bass_utils import run_bass_kernel_spmd
    if "nc" not in _CACHE:
        nc = build_bass()
        nc.finalize()
        _CACHE["nc"] = nc
    in_maps = _make_in_maps(gts, feats)
    return run_bass_kernel_spmd(_CACHE["nc"], in_maps,
                                list(range(NCORES)), trace=trace)


def _combine(results, gts, feats, prev_losses):
    losses = []
    ok = True
    for e in range(E):
        a, b = results[2 * e], results[2 * e + 1]
        Y2 = a["vecs"][0].astype(np.float64) + b["vecs"][0].astype(np.float64)
        ZG = a["vecs"][1].astype(np.float64) + b["vecs"][1].astype(np.float64)
        losses.append(np.sum(ZG / Y2) / (64.0 * N))
        r1 = np.concatenate([a["r1o"].T.reshape(-1), b["r1o"].T.reshape(-1)])
        r2 = np.concatenate([a["r2o"].T.reshape(-1), b["r2o"].T.reshape(-1)])
        err1 = np.mean(np.abs(r1 - 1.0))
        err2 = np.mean(np.abs(r2 - r1))
        if not (err1 >= 0.01 and err2 < 0.01):
            ok = False
    if not ok:
        return _numpy_reference(gts, feats, prev_losses)
    return _ensemble(losses, prev_losses)


def kernel(gts, feats, prev_losses):
    gts = np.asarray(gts, np.float32)
    feats = np.asarray(feats, np.float32)
    prev_losses = np.asarray(prev_losses, np.float32)
    res = _run(gts, feats)
    return _combine(res.results, gts, feats, prev_losses)
